# revision 18
# baseline (speedup 1.0000x reference)
"""Batched pairwise bbox IoU on 8 Trainium2 NeuronCores (Bass/Tile).

Problem: a (4,4096,4) f32, b (4,4096,4) f32 -> IoU (4,4096,4096) f32.

Sharding: 8 cores = 4 batches x 2 column-halves. Core c computes
out[c//2, :, (c%2)*2048 : (c%2+1)*2048] as a (4096, 2048) tile grid,
partition dim = n (32 tiles of 128 rows), free dim = m (2048).

Math per element (all in coordinates pre-scaled by SC=64, so areas scale
by K=4096; the scale cancels in inter'/union'):
  w' = min(ar',br') - max(al',bl') = -( relu(ar'-br') - wa' + relu(bl'-al') )
  h' analogous;  tmp = relu(h') * w'  (signed inter'; relu(tmp) fused into
  later ops via op0=max/min with 0)
  union' = area_a' + eps' + area_b' - relu(tmp)
  IoU = relu(tmp) / union'  via  exp(-ln(max(union', 2e-5)))  on the Scalar
  engine (ACT Reciprocal is banned for accuracy; Ln+Exp share one table set).
  union' < 2e-5 (scaled; 4.9e-9 unscaled) only happens when inter = 0
  (union >= max(area) >= inter when inter > 0; empirical min scaled union
  over inter>0 elements is 4.7e-3), and there out = relu(tmp)*r = 0 for any
  finite r, so the clamp is exact.

Intermediates are fp16 (2x DVE throughput); coordinate differences are
computed from fp32 inputs (fp16 coords lose absolute precision that
cancellation amplifies), and the x64 scaling keeps tiny intersections out
of the fp16 subnormal range. fp16 output is upcast to f32 on the host.

Host-side prep (cheap O(N) layout only): a is permuted so the kernel loads
it with one contiguous DMA; b is transposed to coord-major so the
partition-broadcast DMA uses 128 contiguous 8KB descriptors.
"""

import numpy as np

import concourse.bacc as bacc
import concourse.bass as bass
import concourse.mybir as mybir
import concourse.tile as tile
from concourse.bass_utils import run_bass_kernel_spmd

N_CORES = 8
B, N, M = 4, 4096, 4096
P = 128          # partitions
MW = M // 2      # per-core column width (2048)
NT = N // P      # 32 row tiles per core
SC = 64.0        # coordinate scale; areas scale by SC^2
K2 = SC * SC
EPS = 1e-15
UCLAMP = 2e-5    # union' floor (scaled units); only active where inter = 0

F32 = mybir.dt.float32
F16 = mybir.dt.float16
Alu = mybir.AluOpType
Act = mybir.ActivationFunctionType

_CACHE = {}


def _pin_act_table_set(arch: str):
    """Force every activation we use (Relu/Ln/Exp) to resolve from the one
    table set that contains them all, so the compiled program does a single
    ACT_TABLE_LOAD instead of flip-flopping between sets (~2.7us each).
    get_activation_tables is functools.cached, so in-place mutation sticks;
    set ids stay positional so walrus's id->set mapping is unchanged."""
    from concourse.hw_specs import get_activation_tables
    tables = get_activation_tables(arch)
    keep = "natural_log_exp_and_others"
    if keep not in tables:
        return
    used = {Act.Relu, Act.Ln, Act.Exp, Act.Identity, Act.Copy}
    for name, funcs in tables.items():
        if name != keep:
            funcs -= used


def _build():
    nc = bacc.Bacc("TRN2", target_bir_lowering=False, debug=False,
                   num_devices=N_CORES)
    _pin_act_table_set(nc.m.arch)
    # a: [128 partitions, 32 tiles * 4 coords], host pre-permuted so
    # asc[p, t, c] = a[t*128 + p, c]
    a_d = nc.dram_tensor("a", [P, NT * 4], F32, kind="ExternalInput")
    # b: coord-major [4, MW] (host-transposed slice)
    b_d = nc.dram_tensor("b", [4, MW], F32, kind="ExternalInput")
    o_d = nc.dram_tensor("o", [N, MW], F16, kind="ExternalOutput")

    with tile.TileContext(nc) as tc:
        with (
            tc.tile_pool(name="setup", bufs=1) as setup,
            tc.tile_pool(name="work", bufs=2) as work,
            tc.tile_pool(name="outp", bufs=3) as outp,
        ):
            # ---- per-core a-derived scalars [128, NT] ------------------
            asc_flat = setup.tile([P, NT * 4], F32)
            nc.sync.dma_start(out=asc_flat, in_=a_d.ap())
            ascK = setup.tile([P, NT, 4], F32)
            nc.vector.tensor_scalar(out=ascK,
                                    in0=asc_flat.rearrange("p (t c) -> p t c",
                                                           c=4),
                                    scalar1=SC, scalar2=None, op0=Alu.mult)
            waK = setup.tile([P, NT], F32)
            nc.vector.tensor_tensor(out=waK, in0=ascK[:, :, 2],
                                    in1=ascK[:, :, 0], op=Alu.subtract)
            haK = setup.tile([P, NT], F32)
            nc.vector.tensor_tensor(out=haK, in0=ascK[:, :, 3],
                                    in1=ascK[:, :, 1], op=Alu.subtract)
            areaK = setup.tile([P, NT], F32)
            nc.vector.tensor_tensor(out=areaK, in0=waK, in1=haK, op=Alu.mult)
            SaK = setup.tile([P, NT], F32)
            nc.vector.tensor_scalar(out=SaK, in0=areaK,
                                    scalar1=float(EPS * K2), scalar2=None,
                                    op0=Alu.add)
            negal = setup.tile([P, NT], F32)
            nc.vector.tensor_scalar(out=negal, in0=ascK[:, :, 0], scalar1=-1.0,
                                    scalar2=None, op0=Alu.mult)
            negat = setup.tile([P, NT], F32)
            nc.vector.tensor_scalar(out=negat, in0=ascK[:, :, 1], scalar1=-1.0,
                                    scalar2=None, op0=Alu.mult)

            # ---- b rows broadcast to all partitions, scaled ------------
            bcoord = []
            for c in range(4):
                t = setup.tile([P, MW], F32, tag=f"bco{c}")
                nc.sync.dma_start(
                    out=t,
                    in_=bass.AP(b_d, c * MW, [[0, P], [1, MW]]),
                )
                nc.vector.tensor_scalar(out=t, in0=t, scalar1=SC,
                                        scalar2=None, op0=Alu.mult)
                bcoord.append(t)
            blK, btK, brK, bbK = bcoord
            wbK = setup.tile([P, MW], F32)
            nc.vector.tensor_tensor(out=wbK, in0=brK, in1=blK, op=Alu.subtract)
            hbK = setup.tile([P, MW], F32)
            nc.vector.tensor_tensor(out=hbK, in0=bbK, in1=btK, op=Alu.subtract)
            areab = setup.tile([P, MW], F16)
            nc.vector.tensor_tensor(out=areab, in0=wbK, in1=hbK, op=Alu.mult)

            # ---- main loop over 32 row tiles ---------------------------
            # scalar_tensor_tensor has no fast DVE uop (1x only), so the
            # pipeline uses only ts (2x/4x) + tt (2x) + ACT, balanced so
            # DVE and ACT busy-times roughly match.
            for t in range(NT):
                alK = ascK[:, t, 0:1]
                atK = ascK[:, t, 1:2]

                # corner terms: A2* on ACT, t_* on DVE
                A2w = work.tile([P, MW], F16)
                nc.scalar.activation(out=A2w, in_=blK, func=Act.Relu,
                                     bias=negal[:, t:t + 1], scale=1.0)
                t_w = work.tile([P, MW], F16)
                nc.vector.tensor_scalar(out=t_w, in0=brK, scalar1=alK,
                                        scalar2=waK[:, t:t + 1],
                                        op0=Alu.subtract, op1=Alu.min)
                A2h = work.tile([P, MW], F16)
                nc.scalar.activation(out=A2h, in_=btK, func=Act.Relu,
                                     bias=negat[:, t:t + 1], scale=1.0)
                t_h = work.tile([P, MW], F16)
                nc.vector.tensor_scalar(out=t_h, in0=bbK, scalar1=atK,
                                        scalar2=haK[:, t:t + 1],
                                        op0=Alu.subtract, op1=Alu.min)
                # negw = relu(bl-al) - min(br-al, wa) = -w
                negw = work.tile([P, MW], F16)
                nc.vector.tensor_tensor(out=negw, in0=A2w, in1=t_w,
                                        op=Alu.subtract)
                negh = work.tile([P, MW], F16)
                nc.vector.tensor_tensor(out=negh, in0=A2h, in1=t_h,
                                        op=Alu.subtract)
                rw = work.tile([P, MW], F16)
                nc.vector.tensor_scalar(out=rw, in0=negw, scalar1=-1.0,
                                        scalar2=0.0, op0=Alu.mult, op1=Alu.max)
                rh = work.tile([P, MW], F16)
                if t % 8 < 3:
                    nc.vector.tensor_scalar(out=rh, in0=negh, scalar1=-1.0,
                                            scalar2=0.0, op0=Alu.mult,
                                            op1=Alu.max)
                else:
                    nc.scalar.activation(out=rh, in_=negh, func=Act.Relu,
                                         scale=-1.0)
                inter = work.tile([P, MW], F16)
                nc.vector.tensor_tensor(out=inter, in0=rw, in1=rh, op=Alu.mult)
                u_raw = work.tile([P, MW], F16)
                nc.vector.tensor_tensor(out=u_raw, in0=areab, in1=inter,
                                        op=Alu.subtract)
                u_c = work.tile([P, MW], F16)
                nc.vector.tensor_scalar(out=u_c, in0=u_raw,
                                        scalar1=SaK[:, t:t + 1],
                                        scalar2=UCLAMP, op0=Alu.add,
                                        op1=Alu.max)
                lnu = work.tile([P, MW], F32)
                nc.scalar.activation(out=lnu, in_=u_c, func=Act.Ln)
                rln = work.tile([P, MW], F16)
                nc.scalar.activation(out=rln, in_=lnu, func=Act.Exp,
                                     scale=-1.0)
                ot = outp.tile([P, MW], F16)
                nc.vector.tensor_tensor(out=ot, in0=inter, in1=rln,
                                        op=Alu.mult)
                nc.sync.dma_start(out=o_d.ap()[t * P:(t + 1) * P, :], in_=ot)

    nc.compile()
    return nc


def get_nc():
    if "nc" not in _CACHE:
        _CACHE["nc"] = _build()
    return _CACHE["nc"]


def kernel(a: np.ndarray, b: np.ndarray) -> np.ndarray:
    a = np.asarray(a, dtype=np.float32)
    b = np.asarray(b, dtype=np.float32)
    nc = get_nc()
    in_maps = []
    for c in range(N_CORES):
        bi, half = divmod(c, 2)
        a_perm = np.ascontiguousarray(
            a[bi].reshape(NT, P, 4).transpose(1, 0, 2).reshape(P, NT * 4))
        b_t = np.ascontiguousarray(b[bi, half * MW:(half + 1) * MW].T)
        in_maps.append({"a": a_perm, "b": b_t})
    res = run_bass_kernel_spmd(nc, in_maps, core_ids=list(range(N_CORES)))
    out = np.empty((B, N, M), dtype=np.float32)
    for c in range(N_CORES):
        bi, half = divmod(c, 2)
        out[bi, :, half * MW:(half + 1) * MW] = res.results[c]["o"]
    return out


# revision 19
# speedup vs baseline: 1.0118x; 1.0118x over previous
"""Batched pairwise bbox IoU on 8 Trainium2 NeuronCores (Bass/Tile).

Problem: a (4,4096,4) f32, b (4,4096,4) f32 -> IoU (4,4096,4096) f32.

Sharding: 8 cores = 4 batches x 2 column-halves. Core c computes
out[c//2, :, (c%2)*2048 : (c%2+1)*2048] as a (4096, 2048) tile grid,
partition dim = n (32 tiles of 128 rows), free dim = m (2048).

Math per element (all in coordinates pre-scaled by SC=64, so areas scale
by K=4096; the scale cancels in inter'/union'):
  w' = min(ar',br') - max(al',bl') = -( relu(ar'-br') - wa' + relu(bl'-al') )
  h' analogous;  tmp = relu(h') * w'  (signed inter'; relu(tmp) fused into
  later ops via op0=max/min with 0)
  union' = area_a' + eps' + area_b' - relu(tmp)
  IoU = relu(tmp) / union'  via  exp(-ln(max(union', 2e-5)))  on the Scalar
  engine (ACT Reciprocal is banned for accuracy; Ln+Exp share one table set).
  union' < 2e-5 (scaled; 4.9e-9 unscaled) only happens when inter = 0
  (union >= max(area) >= inter when inter > 0; empirical min scaled union
  over inter>0 elements is 4.7e-3), and there out = relu(tmp)*r = 0 for any
  finite r, so the clamp is exact.

Intermediates are fp16 (2x DVE throughput); coordinate differences are
computed from fp32 inputs (fp16 coords lose absolute precision that
cancellation amplifies), and the x64 scaling keeps tiny intersections out
of the fp16 subnormal range. fp16 output is upcast to f32 on the host.

Host-side prep (cheap O(N) layout only): a is permuted so the kernel loads
it with one contiguous DMA; b is transposed to coord-major so the
partition-broadcast DMA uses 128 contiguous 8KB descriptors.
"""

import numpy as np

import concourse.bacc as bacc
import concourse.bass as bass
import concourse.mybir as mybir
import concourse.tile as tile
from concourse.bass_utils import run_bass_kernel_spmd

N_CORES = 8
B, N, M = 4, 4096, 4096
P = 128          # partitions
MW = M // 2      # per-core column width (2048)
NT = N // P      # 32 row tiles per core
SC = 64.0        # coordinate scale; areas scale by SC^2
K2 = SC * SC
EPS = 1e-15
UCLAMP = 2e-5    # union' floor (scaled units); only active where inter = 0

F32 = mybir.dt.float32
F16 = mybir.dt.float16
Alu = mybir.AluOpType
Act = mybir.ActivationFunctionType

_CACHE = {}


def _pin_act_table_set(arch: str):
    """Force every activation we use (Relu/Ln/Exp) to resolve from the one
    table set that contains them all, so the compiled program does a single
    ACT_TABLE_LOAD instead of flip-flopping between sets (~2.7us each).
    get_activation_tables is functools.cached, so in-place mutation sticks;
    set ids stay positional so walrus's id->set mapping is unchanged."""
    from concourse.hw_specs import get_activation_tables
    tables = get_activation_tables(arch)
    keep = "natural_log_exp_and_others"
    if keep not in tables:
        return
    used = {Act.Relu, Act.Ln, Act.Exp, Act.Identity, Act.Copy}
    for name, funcs in tables.items():
        if name != keep:
            funcs -= used


def _build():
    nc = bacc.Bacc("TRN2", target_bir_lowering=False, debug=False,
                   num_devices=N_CORES)
    _pin_act_table_set(nc.m.arch)
    # a: [128 partitions, 32 tiles * 4 coords], host pre-permuted so
    # asc[p, t, c] = a[t*128 + p, c]
    a_d = nc.dram_tensor("a", [P, NT * 4], F32, kind="ExternalInput")
    # b: coord-major [4, MW] (host-transposed slice)
    b_d = nc.dram_tensor("b", [4, MW], F32, kind="ExternalInput")
    o_d = nc.dram_tensor("o", [N, MW], F16, kind="ExternalOutput")

    with tile.TileContext(nc) as tc:
        with (
            tc.tile_pool(name="setup", bufs=1) as setup,
            tc.tile_pool(name="work", bufs=2) as work,
            tc.tile_pool(name="outp", bufs=3) as outp,
        ):
            # ---- per-core a-derived scalars [128, NT] ------------------
            asc_flat = setup.tile([P, NT * 4], F32)
            nc.sync.dma_start(out=asc_flat, in_=a_d.ap())
            ascK = setup.tile([P, NT, 4], F32)
            nc.vector.tensor_scalar(out=ascK,
                                    in0=asc_flat.rearrange("p (t c) -> p t c",
                                                           c=4),
                                    scalar1=SC, scalar2=None, op0=Alu.mult)
            waK = setup.tile([P, NT], F32)
            nc.vector.tensor_tensor(out=waK, in0=ascK[:, :, 2],
                                    in1=ascK[:, :, 0], op=Alu.subtract)
            haK = setup.tile([P, NT], F32)
            nc.vector.tensor_tensor(out=haK, in0=ascK[:, :, 3],
                                    in1=ascK[:, :, 1], op=Alu.subtract)
            areaK = setup.tile([P, NT], F32)
            nc.vector.tensor_tensor(out=areaK, in0=waK, in1=haK, op=Alu.mult)
            SaK = setup.tile([P, NT], F32)
            nc.vector.tensor_scalar(out=SaK, in0=areaK,
                                    scalar1=float(EPS * K2), scalar2=None,
                                    op0=Alu.add)
            negal = setup.tile([P, NT], F32)
            nc.vector.tensor_scalar(out=negal, in0=ascK[:, :, 0], scalar1=-1.0,
                                    scalar2=None, op0=Alu.mult)
            negat = setup.tile([P, NT], F32)
            nc.vector.tensor_scalar(out=negat, in0=ascK[:, :, 1], scalar1=-1.0,
                                    scalar2=None, op0=Alu.mult)

            # ---- b rows broadcast to all partitions, scaled ------------
            bcoord = []
            for c in range(4):
                t = setup.tile([P, MW], F32, tag=f"bco{c}")
                nc.sync.dma_start(
                    out=t,
                    in_=bass.AP(b_d, c * MW, [[0, P], [1, MW]]),
                )
                nc.vector.tensor_scalar(out=t, in0=t, scalar1=SC,
                                        scalar2=None, op0=Alu.mult)
                bcoord.append(t)
            blK, btK, brK, bbK = bcoord
            wbK = setup.tile([P, MW], F32)
            nc.vector.tensor_tensor(out=wbK, in0=brK, in1=blK, op=Alu.subtract)
            hbK = setup.tile([P, MW], F32)
            nc.vector.tensor_tensor(out=hbK, in0=bbK, in1=btK, op=Alu.subtract)
            areab = setup.tile([P, MW], F16)
            nc.vector.tensor_tensor(out=areab, in0=wbK, in1=hbK, op=Alu.mult)

            # ---- main loop over 32 row tiles ---------------------------
            # scalar_tensor_tensor has no fast DVE uop (1x only), so the
            # pipeline uses only ts (2x/4x) + tt (2x) + ACT, balanced so
            # DVE and ACT busy-times roughly match.
            for t in range(NT):
                alK = ascK[:, t, 0:1]
                atK = ascK[:, t, 1:2]

                # corner terms: A2* on ACT, t_* on DVE
                A2w = work.tile([P, MW], F16)
                nc.scalar.activation(out=A2w, in_=blK, func=Act.Relu,
                                     bias=negal[:, t:t + 1], scale=1.0)
                t_w = work.tile([P, MW], F16)
                nc.vector.tensor_scalar(out=t_w, in0=brK, scalar1=alK,
                                        scalar2=waK[:, t:t + 1],
                                        op0=Alu.subtract, op1=Alu.min)
                A2h = work.tile([P, MW], F16)
                nc.scalar.activation(out=A2h, in_=btK, func=Act.Relu,
                                     bias=negat[:, t:t + 1], scale=1.0)
                t_h = work.tile([P, MW], F16)
                nc.vector.tensor_scalar(out=t_h, in0=bbK, scalar1=atK,
                                        scalar2=haK[:, t:t + 1],
                                        op0=Alu.subtract, op1=Alu.min)
                # negw = relu(bl-al) - min(br-al, wa) = -w
                negw = work.tile([P, MW], F16)
                nc.vector.tensor_tensor(out=negw, in0=A2w, in1=t_w,
                                        op=Alu.subtract)
                negh = work.tile([P, MW], F16)
                nc.vector.tensor_tensor(out=negh, in0=A2h, in1=t_h,
                                        op=Alu.subtract)
                rw = work.tile([P, MW], F16)
                nc.vector.tensor_scalar(out=rw, in0=negw, scalar1=-1.0,
                                        scalar2=0.0, op0=Alu.mult, op1=Alu.max)
                rh = work.tile([P, MW], F16)
                if t % 8 == 0:
                    nc.vector.tensor_scalar(out=rh, in0=negh, scalar1=-1.0,
                                            scalar2=0.0, op0=Alu.mult,
                                            op1=Alu.max)
                else:
                    nc.scalar.activation(out=rh, in_=negh, func=Act.Relu,
                                         scale=-1.0)
                inter = work.tile([P, MW], F16)
                nc.vector.tensor_tensor(out=inter, in0=rw, in1=rh, op=Alu.mult)
                u_raw = work.tile([P, MW], F16)
                nc.vector.tensor_tensor(out=u_raw, in0=areab, in1=inter,
                                        op=Alu.subtract)
                u_c = work.tile([P, MW], F16)
                nc.vector.tensor_scalar(out=u_c, in0=u_raw,
                                        scalar1=SaK[:, t:t + 1],
                                        scalar2=UCLAMP, op0=Alu.add,
                                        op1=Alu.max)
                lnu = work.tile([P, MW], F32)
                nc.scalar.activation(out=lnu, in_=u_c, func=Act.Ln)
                rln = work.tile([P, MW], F16)
                nc.scalar.activation(out=rln, in_=lnu, func=Act.Exp,
                                     scale=-1.0)
                ot = outp.tile([P, MW], F16)
                nc.vector.tensor_tensor(out=ot, in0=inter, in1=rln,
                                        op=Alu.mult)
                nc.sync.dma_start(out=o_d.ap()[t * P:(t + 1) * P, :], in_=ot)

    nc.compile()
    return nc


def get_nc():
    if "nc" not in _CACHE:
        _CACHE["nc"] = _build()
    return _CACHE["nc"]


def kernel(a: np.ndarray, b: np.ndarray) -> np.ndarray:
    a = np.asarray(a, dtype=np.float32)
    b = np.asarray(b, dtype=np.float32)
    nc = get_nc()
    in_maps = []
    for c in range(N_CORES):
        bi, half = divmod(c, 2)
        a_perm = np.ascontiguousarray(
            a[bi].reshape(NT, P, 4).transpose(1, 0, 2).reshape(P, NT * 4))
        b_t = np.ascontiguousarray(b[bi, half * MW:(half + 1) * MW].T)
        in_maps.append({"a": a_perm, "b": b_t})
    res = run_bass_kernel_spmd(nc, in_maps, core_ids=list(range(N_CORES)))
    out = np.empty((B, N, M), dtype=np.float32)
    for c in range(N_CORES):
        bi, half = divmod(c, 2)
        out[bi, :, half * MW:(half + 1) * MW] = res.results[c]["o"]
    return out
